# revision 10
# baseline (speedup 1.0000x reference)
"""Trainium2 Bass kernel for nn_HRNetW30classifier: logits = x @ W.T + b.

Shapes (full): x (8192, 2048) f32, W (1000, 2048) f32, b (1000,) f32
Output: (8192, 1000) f32.

Sharding: data-parallel over batch across 8 NeuronCores. Each core computes a
(1024, 2048) @ (2048, 1000) GEMM with W/b replicated.

Numerics: hybrid fp8/fp16. Features [0:K1=512] run as e4m3 DoubleRow matmuls
(2 k-rows per partition, 2x PE throughput); features [K1:2048] run in fp16.
Host pre-scales so one PSUM accumulation carries both parts at a common 256x
scale (x8 = e4m3(4x), w8 = e4m3(64W), w16 = fp16(256W) -- all pow-2 exact),
and the final /256 is an exact host-side multiply after the gather. Measured
rel err vs the fp32 reference: 1.54e-2 (fp16-only: 2.4e-4).

Schedule (from NTFF trace analysis):
- Heavy warmups: full 128-partition x 256-col dummy matmuls from t~6us open
  the HAM clock window ~5.8us later, so real matmuls run at the full
  2.4GHz (0.411 ns/col) instead of the ~2GHz a cold PE sustains.
- Inputs ride BOTH hardware DGE queues (Sync: W, Activation: x) in
  need-order: fp8 k-slices first, then per-kt fp16 phase-1 halves, then the
  batched phase-2 half.
- Phase 1 (m-tiles 0-3) runs k-outer paced by the stream; phase 2 runs
  group-serial per m-tile so evictions stagger; the last m-tile runs its two
  N-chunks serially with sliced evictions alternating the two DGE queues so
  only ~1us trails the final matmul.
"""

import numpy as np

P = 128
N_CORES = 8
B_FULL = 8192
M = B_FULL // N_CORES  # 1024 batch rows per core
N = 1000  # classes
K = 2048  # features
K1 = 512  # features computed in fp8 DoubleRow
K2 = K - K1  # 1536 features computed in fp16
KT8 = K1 // (2 * P)  # 2 double-k-tiles
KT16 = K2 // P  # 12 fp16 k-tiles
MT = M // P  # 8 m-tiles
MH = MT // 2  # 4 m-tiles per phase
MH_COLS = MH * P  # 512 x columns needed for phase 1
N0_W = 512  # first n-chunk (one PSUM bank of fp32)
N1_W = N - N0_W  # 488
SX = 4.0  # host scale on x before e4m3 quantization
SW8 = 64.0  # host scale on W[:, :K1] before e4m3 quantization
SCALE = SX * SW8  # PSUM carries SCALE * logits
INV_SCALE = np.float32(1.0 / SCALE)  # exact pow-2 host dequant

WARM_N = 15  # heavy warmup matmuls: bridge PE activity from preamble end
# (~7.2us, ~214ns each) until the first k-slice lands (~10us). The HAM
# full-clock window opens after ~4us of CONTINUOUS heavy activity; any PE
# idle gap before real matmuls restarts that ramp (costs ~2.5us), so err on
# the long side -- extra warmups only cost ~0.2us each.

_NC_CACHE = {}


def _build_nc():
    """Build + compile the per-core Bass program (SPMD: same NEFF on 8 cores)."""
    from contextlib import ExitStack

    import concourse.tile as tile
    from concourse import bacc, mybir
    from concourse._compat import get_trn_type

    f32 = mybir.dt.float32
    f16 = mybir.dt.float16
    f8 = mybir.dt.float8e4
    DR = mybir.MatmulPerfMode.DoubleRow

    nc = bacc.Bacc(get_trn_type() or "TRN2", target_bir_lowering=False, debug=False)

    # fp8 operands, host-interleaved for DoubleRow: [kt2][p][i*cols + c] holds
    # feature k = kt2*256 + i*128 + p.
    x8T = nc.dram_tensor("x8T", [KT8, P, 2 * M], f8, kind="ExternalInput")
    w8T = nc.dram_tensor("w8T", [KT8, P, 2 * N], f8, kind="ExternalInput")
    # fp16 operands for features K1..K, K-major.
    x16T = nc.dram_tensor("x16T", [K2, M], f16, kind="ExternalInput")
    w16T = nc.dram_tensor("w16T", [K2, N], f16, kind="ExternalInput")
    bias = nc.dram_tensor("bias", [P, N], f32, kind="ExternalInput")  # SCALE*b
    out = nc.dram_tensor("out", [M, N], f32, kind="ExternalOutput")  # SCALE*logits

    x16_r = x16T.ap().rearrange("(kt p) m -> kt p m", p=P)  # [KT16, 128, M]
    x16_p = x16T.ap().rearrange("(kt p) m -> p kt m", p=P)  # [128, KT16, M]
    w16_r = w16T.ap().rearrange("(kt p) n -> kt p n", p=P)  # [KT16, 128, N]
    out_r = out.ap().rearrange("(mt p) n -> mt p n", p=P)  # [MT, 128, N]

    with tile.TileContext(nc) as tc:
        with ExitStack() as ctx:
            xpool = ctx.enter_context(tc.tile_pool(name="xpool", bufs=1))
            wpool = ctx.enter_context(tc.tile_pool(name="wpool", bufs=1))
            bpool = ctx.enter_context(tc.tile_pool(name="bpool", bufs=1))
            opool = ctx.enter_context(tc.tile_pool(name="opool", bufs=8))
            pspool = ctx.enter_context(tc.tile_pool(name="ps", bufs=8, space="PSUM"))

            x8_sb = xpool.tile([P, KT8, 2, M], f8, tag="x8")
            x16_sb = xpool.tile([P, KT16, M], f16, tag="x16")
            w8_sb = wpool.tile([P, KT8, 2, N], f8, tag="w8")
            w16_sb = wpool.tile([P, KT16, N], f16, tag="w16")
            warm = bpool.tile([P, 384], f16, tag="warm")
            bias_t = bpool.tile([P, N], f32, tag="bias")

            # Input DMA streams on BOTH hardware DGE queues, in need-order.
            # Sync queue: W (fp8 slices then fp16 k-slices, 2000B lines).
            # Activation queue: x (fp8 slices, fp16 phase-1 halves, bias,
            # then the whole fp16 phase-2 half as one batched 3D DMA).
            # fp16 kt=0 leads (it opens every PSUM bank with start=True),
            # then the fp8 slices, then the remaining fp16 k-slices.
            nc.scalar.dma_start(x16_sb[:, 0, 0:P], x16_r[0][:, 0:P])
            nc.sync.dma_start(w16_sb[:, 0, 0:N0_W], w16_r[0][:, 0:N0_W])
            nc.scalar.dma_start(x16_sb[:, 0, P:MH_COLS], x16_r[0][:, P:MH_COLS])
            nc.sync.dma_start(w16_sb[:, 0, N0_W:N], w16_r[0][:, N0_W:N])
            for kt2 in range(KT8):
                nc.scalar.dma_start(x8_sb[:, kt2, :, :], x8T.ap()[kt2])
                nc.sync.dma_start(w8_sb[:, kt2, :, :], w8T.ap()[kt2])
            for kt in range(1, KT16):
                nc.sync.dma_start(w16_sb[:, kt, :], w16_r[kt])
                nc.scalar.dma_start(x16_sb[:, kt, 0:MH_COLS], x16_r[kt][:, 0:MH_COLS])
                if kt == 3:
                    # bias rides early-mid stream: needed by the first
                    # evictions (~35us).
                    nc.scalar.dma_start(bias_t[:], bias.ap())
            # fp16 phase-2 x halves: one 1.5MB batched DMA.
            nc.scalar.dma_start(x16_sb[:, :, MH_COLS:M], x16_p[:, :, MH_COLS:M])

            # Heavy warmups: full 128-partition stationary tile and 256-col
            # moving tile, so the HAM activity monitor sees real PE load and
            # opens the full-clock window before the first data-dependent
            # matmul. Results go to a scratch PSUM bank that is never read.
            nc.gpsimd.memset(warm[:], 0.7071)
            ps_w = pspool.tile([P, N0_W], f32, tag="ps", name="ps_warm")
            for _ in range(WARM_N):
                nc.tensor.matmul(
                    ps_w[:, :256],
                    lhsT=warm[:, 0:P],
                    rhs=warm[:, P:384],
                    start=True,
                    stop=True,
                )

            # fp8 n-chunks: DoubleRow moving free dim is 2*cw <= 512.
            F8_CHUNKS_A = [(0, 256), (256, 256)]  # psA covers n 0:512
            F8_CHUNKS_B = [(512, 256), (768, 232)]  # psB covers n 512:1000

            # HW start=True zeroes PSUM at bank granularity (wider than the
            # instruction's write region), so each bank must be opened by
            # exactly one full-width matmul: the fp16 kt=0 chunk. All fp8
            # DoubleRow chunks accumulate with start=False.
            def mm_f8(psA, psB, mt, kt2):
                lhsT = x8_sb[:, kt2, :, mt * P : (mt + 1) * P]
                for n0, cw in F8_CHUNKS_A:
                    nc.tensor.matmul(
                        psA[:, n0 : n0 + cw],
                        lhsT=lhsT,
                        rhs=w8_sb[:, kt2, :, n0 : n0 + cw],
                        start=False,
                        stop=False,
                        perf_mode=DR,
                    )
                for n0, cw in F8_CHUNKS_B:
                    nc.tensor.matmul(
                        psB[:, n0 - N0_W : n0 - N0_W + cw],
                        lhsT=lhsT,
                        rhs=w8_sb[:, kt2, :, n0 : n0 + cw],
                        start=False,
                        stop=False,
                        perf_mode=DR,
                    )

            def mm_f16(psA, psB, mt, kt, start, stop):
                lhsT = x16_sb[:, kt, mt * P : (mt + 1) * P]
                nc.tensor.matmul(
                    psA[:, :N0_W],
                    lhsT=lhsT,
                    rhs=w16_sb[:, kt, 0:N0_W],
                    start=start,
                    stop=stop,
                )
                nc.tensor.matmul(
                    psB[:, :N1_W],
                    lhsT=lhsT,
                    rhs=w16_sb[:, kt, N0_W:N],
                    start=start,
                    stop=stop,
                )

            _evict_i = [0]

            def evict(ps_t, mt, n0, nw, slices=1):
                """Bias-add + store, optionally sliced for tail pipelining.
                Adds run on vector (gpsimd cannot read PSUM); DMAs alternate
                the two DGE queues so the tail drains in parallel."""
                ot = opool.tile([P, N0_W], f32, tag="ot", name=f"ot_{n0}_{mt}")
                step = -(-nw // slices)
                for s0 in range(0, nw, step):
                    sw = min(step, nw - s0)
                    i = _evict_i[0]
                    _evict_i[0] += 1
                    dma_eng = nc.sync if i % 2 == 0 else nc.scalar
                    nc.vector.tensor_add(
                        ot[:, s0 : s0 + sw],
                        ps_t[:, s0 : s0 + sw],
                        bias_t[:, n0 + s0 : n0 + s0 + sw],
                    )
                    dma_eng.dma_start(
                        out_r[mt, :, n0 + s0 : n0 + s0 + sw], ot[:, s0 : s0 + sw]
                    )

            def ps_pair(mt):
                a = pspool.tile([P, N0_W], f32, tag="ps", name=f"psA_{mt}")
                b = pspool.tile([P, N0_W], f32, tag="ps", name=f"psB_{mt}")
                return a, b

            # ---- phase 1: mt 0..3, k-outer, paced by the DMA stream ----
            ps1 = [ps_pair(mt) for mt in range(MH)]
            for mt in range(MH):
                mm_f16(*ps1[mt], mt, 0, start=True, stop=False)
            for kt2 in range(KT8):
                for mt in range(MH):
                    mm_f8(*ps1[mt], mt, kt2)
            for kt in range(1, KT16):
                for mt in range(MH):
                    mm_f16(*ps1[mt], mt, kt, start=False, stop=(kt == KT16 - 1))
            for mt in range(MH):
                evict(ps1[mt][0], mt, 0, N0_W)
                evict(ps1[mt][1], mt, N0_W, N1_W)

            # ---- phase 2: mt 4..6 group-serial so evictions stagger ----
            for mt in range(MH, MT - 1):
                psA, psB = ps_pair(mt)
                mm_f16(psA, psB, mt, 0, start=True, stop=False)
                for kt2 in range(KT8):
                    mm_f8(psA, psB, mt, kt2)
                for kt in range(1, KT16):
                    mm_f16(psA, psB, mt, kt, start=False, stop=(kt == KT16 - 1))
                evict(psA, mt, 0, N0_W)
                evict(psB, mt, N0_W, N1_W)

            # ---- phase 2 tail: last mt fully group-serial; the N0 chunk's
            # eviction overlaps the N1 chunk's matmuls, and the final N1
            # eviction is sliced so only ~1us trails the last matmul.
            mt = MT - 1
            psA, psB = ps_pair(mt)
            nc.tensor.matmul(
                psA[:, :N0_W],
                lhsT=x16_sb[:, 0, mt * P : (mt + 1) * P],
                rhs=w16_sb[:, 0, 0:N0_W],
                start=True,
                stop=False,
            )
            for kt2 in range(KT8):
                lhsT = x8_sb[:, kt2, :, mt * P : (mt + 1) * P]
                for n0, cw in F8_CHUNKS_A:
                    nc.tensor.matmul(
                        psA[:, n0 : n0 + cw],
                        lhsT=lhsT,
                        rhs=w8_sb[:, kt2, :, n0 : n0 + cw],
                        start=False,
                        stop=False,
                        perf_mode=DR,
                    )
            for kt in range(1, KT16):
                nc.tensor.matmul(
                    psA[:, :N0_W],
                    lhsT=x16_sb[:, kt, mt * P : (mt + 1) * P],
                    rhs=w16_sb[:, kt, 0:N0_W],
                    start=False,
                    stop=(kt == KT16 - 1),
                )
            evict(psA, mt, 0, N0_W, slices=2)
            nc.tensor.matmul(
                psB[:, :N1_W],
                lhsT=x16_sb[:, 0, mt * P : (mt + 1) * P],
                rhs=w16_sb[:, 0, N0_W:N],
                start=True,
                stop=False,
            )
            for kt2 in range(KT8):
                lhsT = x8_sb[:, kt2, :, mt * P : (mt + 1) * P]
                for n0, cw in F8_CHUNKS_B:
                    nc.tensor.matmul(
                        psB[:, n0 - N0_W : n0 - N0_W + cw],
                        lhsT=lhsT,
                        rhs=w8_sb[:, kt2, :, n0 : n0 + cw],
                        start=False,
                        stop=False,
                        perf_mode=DR,
                    )
            for kt in range(1, KT16):
                nc.tensor.matmul(
                    psB[:, :N1_W],
                    lhsT=x16_sb[:, kt, mt * P : (mt + 1) * P],
                    rhs=w16_sb[:, kt, N0_W:N],
                    start=False,
                    stop=(kt == KT16 - 1),
                )
            evict(psB, mt, N0_W, N1_W, slices=2)

    nc.compile()
    return nc


def _get_nc():
    if "nc" not in _NC_CACHE:
        _NC_CACHE["nc"] = _build_nc()
    return _NC_CACHE["nc"]


def _run(in_maps, trace=False, **kwargs):
    from concourse.bass_utils import run_bass_kernel_spmd

    nc = _get_nc()
    return run_bass_kernel_spmd(
        nc, in_maps, core_ids=list(range(N_CORES)), trace=trace, **kwargs
    )


def _interleave_f8(aT):
    """[K1, cols] K-major fp8 -> [KT8, P, 2*cols] DoubleRow layout:
    out[kt2][p][i*cols + c] = aT[kt2*256 + i*128 + p, c]."""
    cols = aT.shape[1]
    return np.ascontiguousarray(
        aT.reshape(KT8, 2, P, cols).transpose(0, 2, 1, 3).reshape(KT8, P, 2 * cols)
    )


def _make_in_maps(x, W, b):
    import ml_dtypes

    e4m3 = ml_dtypes.float8_e4m3
    x = np.asarray(x, dtype=np.float32)
    W = np.asarray(W, dtype=np.float32)
    b = np.asarray(b, dtype=np.float32)

    w8 = (W[:, :K1].T * np.float32(SW8)).astype(e4m3)  # [K1, N]
    w16 = (W[:, K1:].T * np.float32(SCALE)).astype(np.float16)  # [K2, N]
    w8T = _interleave_f8(w8)
    bias = np.ascontiguousarray(
        np.broadcast_to((b * np.float32(SCALE))[None, :], (P, N))
    )
    x8_full = (x[:, :K1].T * np.float32(SX)).astype(e4m3)  # [K1, B]
    x16_full = np.ascontiguousarray(x[:, K1:].T).astype(np.float16)  # [K2, B]
    return [
        {
            "x8T": _interleave_f8(x8_full[:, c * M : (c + 1) * M]),
            "x16T": np.ascontiguousarray(x16_full[:, c * M : (c + 1) * M]),
            "w8T": w8T,
            "w16T": w16,
            "bias": bias,
        }
        for c in range(N_CORES)
    ]


def kernel(x, W, b):
    res = _run(_make_in_maps(x, W, b))
    return np.concatenate(
        [r["out"].astype(np.float32) * INV_SCALE for r in res.results], axis=0
    )


# revision 18
# speedup vs baseline: 1.1481x; 1.1481x over previous
"""Trainium2 Bass kernel for nn_HRNetW30classifier: logits = x @ W.T + b.

Shapes (full): x (8192, 2048) f32, W (1000, 2048) f32, b (1000,) f32
Output: (8192, 1000) f32.

Sharding: data-parallel over batch across 8 NeuronCores. Each core computes a
(1024, 2048) @ (2048, 1000) GEMM with W/b replicated.

Numerics: hybrid fp8/fp16. Features [0:K1=512] run as e4m3 DoubleRow matmuls
(2 k-rows per partition, 2x PE throughput); features [K1:2048] run in fp16.
Host pre-scales so one PSUM accumulation carries both parts at a common 256x
scale (x8 = e4m3(4x), w8 = e4m3(64W), w16 = fp16(256W) -- all pow-2 exact),
and the final /256 is an exact host-side multiply after the gather. Measured
rel err vs the fp32 reference: 1.54e-2 (fp16-only: 2.4e-4).

Schedule (from NTFF trace analysis):
- Heavy warmups: full 128-partition x 256-col dummy matmuls from t~6us open
  the HAM clock window ~5.8us later, so real matmuls run at the full
  2.4GHz (0.411 ns/col) instead of the ~2GHz a cold PE sustains.
- Inputs ride BOTH hardware DGE queues (Sync: W, Activation: x) in
  need-order: fp8 k-slices first, then per-kt fp16 phase-1 halves, then the
  batched phase-2 half.
- Phase 1 (m-tiles 0-3) runs k-outer paced by the stream; phase 2 runs
  group-serial per m-tile so evictions stagger; the last m-tile runs its two
  N-chunks serially with sliced evictions alternating the two DGE queues so
  only ~1us trails the final matmul.
"""

import numpy as np

P = 128
N_CORES = 8
B_FULL = 8192
M = B_FULL // N_CORES  # 1024 batch rows per core
N = 1000  # classes
K = 2048  # features
K1 = 512  # features computed in fp8 DoubleRow
K2 = K - K1  # 1536 features computed in fp16
KT8 = K1 // (2 * P)  # 2 double-k-tiles
KT16 = K2 // P  # 12 fp16 k-tiles
MT = M // P  # 8 m-tiles
MH = MT // 2  # 4 m-tiles per phase
MH_COLS = MH * P  # 512 x columns needed for phase 1
N0_W = 512  # first n-chunk (one PSUM bank of fp32)
N1_W = N - N0_W  # 488
SX = 4.0  # host scale on x before e4m3 quantization
SW8 = 64.0  # host scale on W[:, :K1] before e4m3 quantization
SCALE = SX * SW8  # PSUM carries SCALE * logits
INV_SCALE = np.float32(1.0 / SCALE)  # exact pow-2 host dequant

WARM_N = 15  # heavy warmup matmuls: bridge PE activity from preamble end
# (~7.2us, ~214ns each) until the first k-slice lands (~10us). The HAM
# full-clock window opens after ~4us of CONTINUOUS heavy activity; any PE
# idle gap before real matmuls restarts that ramp (costs ~2.5us), so err on
# the long side -- extra warmups only cost ~0.2us each.

_NC_CACHE = {}


def _build_nc():
    """Build + compile the per-core Bass program (SPMD: same NEFF on 8 cores)."""
    from contextlib import ExitStack

    import concourse.tile as tile
    from concourse import bacc, mybir
    from concourse._compat import get_trn_type

    f32 = mybir.dt.float32
    f16 = mybir.dt.float16
    f8 = mybir.dt.float8e4
    DR = mybir.MatmulPerfMode.DoubleRow

    nc = bacc.Bacc(get_trn_type() or "TRN2", target_bir_lowering=False, debug=False)

    # fp8 operands, host-interleaved for DoubleRow: [kt2][p][i*cols + c] holds
    # feature k = kt2*256 + i*128 + p.
    x8T = nc.dram_tensor("x8T", [KT8, P, 2 * M], f8, kind="ExternalInput")
    w8T = nc.dram_tensor("w8T", [KT8, P, 2 * N], f8, kind="ExternalInput")
    # fp16 operands for features K1..K, K-major.
    x16T = nc.dram_tensor("x16T", [K2, M], f16, kind="ExternalInput")
    w16T = nc.dram_tensor("w16T", [K2, N], f16, kind="ExternalInput")
    bias = nc.dram_tensor("bias", [P, N], f32, kind="ExternalInput")  # SCALE*b
    bias16 = nc.dram_tensor("bias16", [1, N], f16, kind="ExternalInput")  # SCALE*b
    out = nc.dram_tensor("out", [M, N], f32, kind="ExternalOutput")  # SCALE*logits

    x16_r = x16T.ap().rearrange("(kt p) m -> kt p m", p=P)  # [KT16, 128, M]
    x16_p = x16T.ap().rearrange("(kt p) m -> p kt m", p=P)  # [128, KT16, M]
    w16_r = w16T.ap().rearrange("(kt p) n -> kt p n", p=P)  # [KT16, 128, N]
    out_r = out.ap().rearrange("(mt p) n -> mt p n", p=P)  # [MT, 128, N]

    with tile.TileContext(nc) as tc:
        with ExitStack() as ctx:
            xpool = ctx.enter_context(tc.tile_pool(name="xpool", bufs=1))
            wpool = ctx.enter_context(tc.tile_pool(name="wpool", bufs=1))
            bpool = ctx.enter_context(tc.tile_pool(name="bpool", bufs=1))
            opool = ctx.enter_context(tc.tile_pool(name="opool", bufs=8))
            pspool = ctx.enter_context(tc.tile_pool(name="ps", bufs=8, space="PSUM"))

            x8_sb = xpool.tile([P, KT8, 2, M], f8, tag="x8")
            x16_sb = xpool.tile([P, KT16, M], f16, tag="x16")
            w8_sb = wpool.tile([P, KT8, 2, N], f8, tag="w8")
            w16_sb = wpool.tile([P, KT16, N], f16, tag="w16")
            warm = bpool.tile([P, 384], f16, tag="warm")
            bias_t = bpool.tile([P, N], f32, tag="bias")
            ones_t = bpool.tile([1, P], f16, tag="ones")
            bias16_t = bpool.tile([1, N], f16, tag="bias16")

            # Input DMA streams on BOTH hardware DGE queues, in need-order.
            # Sync queue: W (fp8 slices then fp16 k-slices, 2000B lines).
            # Activation queue: x (fp8 slices, fp16 phase-1 halves, bias,
            # then the whole fp16 phase-2 half as one batched 3D DMA).
            # fp16 kt=0 leads (it opens every PSUM bank with start=True),
            # then the fp8 slices, then the remaining fp16 k-slices.
            nc.scalar.dma_start(x16_sb[:, 0, 0:P], x16_r[0][:, 0:P])
            nc.sync.dma_start(w16_sb[:, 0, 0:N0_W], w16_r[0][:, 0:N0_W])
            nc.scalar.dma_start(x16_sb[:, 0, P:MH_COLS], x16_r[0][:, P:MH_COLS])
            nc.sync.dma_start(w16_sb[:, 0, N0_W:N], w16_r[0][:, N0_W:N])
            for kt2 in range(KT8):
                nc.scalar.dma_start(x8_sb[:, kt2, :, :], x8T.ap()[kt2])
                nc.sync.dma_start(w8_sb[:, kt2, :, :], w8T.ap()[kt2])
            for kt in range(1, KT16):
                nc.sync.dma_start(w16_sb[:, kt, :], w16_r[kt])
                nc.scalar.dma_start(x16_sb[:, kt, 0:MH_COLS], x16_r[kt][:, 0:MH_COLS])
                if kt == 3:
                    # bias rides early-mid stream: needed by the first
                    # evictions (~35us).
                    nc.scalar.dma_start(bias_t[:], bias.ap())
                    nc.scalar.dma_start(bias16_t[:], bias16.ap())
            # fp16 phase-2 x halves: one 1.5MB batched DMA.
            nc.scalar.dma_start(x16_sb[:, :, MH_COLS:M], x16_p[:, :, MH_COLS:M])

            # Heavy warmups: full 128-partition stationary tile and 256-col
            # moving tile, so the HAM activity monitor sees real PE load and
            # opens the full-clock window before the first data-dependent
            # matmul. Results go to a scratch PSUM bank that is never read.
            nc.gpsimd.memset(warm[:], 0.7071)
            nc.gpsimd.memset(ones_t[:], 1.0)
            ps_w = pspool.tile([P, N0_W], f32, tag="ps", name="ps_warm")
            for _ in range(WARM_N):
                nc.tensor.matmul(
                    ps_w[:, :256],
                    lhsT=warm[:, 0:P],
                    rhs=warm[:, P:384],
                    start=True,
                    stop=True,
                )

            # fp8 n-chunks: DoubleRow moving free dim is 2*cw <= 512.
            F8_CHUNKS_A = [(0, 256), (256, 256)]  # psA covers n 0:512
            F8_CHUNKS_B = [(512, 256), (768, 232)]  # psB covers n 512:1000

            # HW start=True zeroes PSUM at bank granularity (wider than the
            # instruction's write region), so each bank must be opened by
            # exactly one full-width matmul: the fp16 kt=0 chunk. All fp8
            # DoubleRow chunks accumulate with start=False.
            def mm_f8(psA, psB, mt, kt2):
                lhsT = x8_sb[:, kt2, :, mt * P : (mt + 1) * P]
                for n0, cw in F8_CHUNKS_A:
                    nc.tensor.matmul(
                        psA[:, n0 : n0 + cw],
                        lhsT=lhsT,
                        rhs=w8_sb[:, kt2, :, n0 : n0 + cw],
                        start=False,
                        stop=False,
                        perf_mode=DR,
                    )
                for n0, cw in F8_CHUNKS_B:
                    nc.tensor.matmul(
                        psB[:, n0 - N0_W : n0 - N0_W + cw],
                        lhsT=lhsT,
                        rhs=w8_sb[:, kt2, :, n0 : n0 + cw],
                        start=False,
                        stop=False,
                        perf_mode=DR,
                    )

            def mm_f16(psA, psB, mt, kt, start, stop):
                lhsT = x16_sb[:, kt, mt * P : (mt + 1) * P]
                nc.tensor.matmul(
                    psA[:, :N0_W],
                    lhsT=lhsT,
                    rhs=w16_sb[:, kt, 0:N0_W],
                    start=start,
                    stop=stop,
                )
                nc.tensor.matmul(
                    psB[:, :N1_W],
                    lhsT=lhsT,
                    rhs=w16_sb[:, kt, N0_W:N],
                    start=start,
                    stop=stop,
                )

            _evict_i = [0]

            def evict(ps_t, mt, n0, nw, slices=1):
                """Bias-add + store, optionally sliced for tail pipelining.
                Adds run on vector (gpsimd cannot read PSUM); DMAs alternate
                the two DGE queues so the tail drains in parallel."""
                ot = opool.tile([P, N0_W], f32, tag="ot", name=f"ot_{n0}_{mt}")
                step = -(-nw // slices)
                for s0 in range(0, nw, step):
                    sw = min(step, nw - s0)
                    i = _evict_i[0]
                    _evict_i[0] += 1
                    dma_eng = nc.sync if i % 2 == 0 else nc.scalar
                    nc.vector.tensor_add(
                        ot[:, s0 : s0 + sw],
                        ps_t[:, s0 : s0 + sw],
                        bias_t[:, n0 + s0 : n0 + s0 + sw],
                    )
                    dma_eng.dma_start(
                        out_r[mt, :, n0 + s0 : n0 + s0 + sw], ot[:, s0 : s0 + sw]
                    )

            def ps_pair(mt):
                a = pspool.tile([P, N0_W], f32, tag="ps", name=f"psA_{mt}")
                b = pspool.tile([P, N0_W], f32, tag="ps", name=f"psB_{mt}")
                return a, b

            # ---- phase 1: mt 0..3, k-outer, paced by the DMA stream ----
            ps1 = [ps_pair(mt) for mt in range(MH)]
            for mt in range(MH):
                mm_f16(*ps1[mt], mt, 0, start=True, stop=False)
            for kt2 in range(KT8):
                for mt in range(MH):
                    mm_f8(*ps1[mt], mt, kt2)
            for kt in range(1, KT16):
                for mt in range(MH):
                    mm_f16(*ps1[mt], mt, kt, start=False, stop=(kt == KT16 - 1))
            for mt in range(MH):
                evict(ps1[mt][0], mt, 0, N0_W)
                evict(ps1[mt][1], mt, N0_W, N1_W)

            # ---- phase 2: mt 4..6 group-serial so evictions stagger ----
            for mt in range(MH, MT - 1):
                psA, psB = ps_pair(mt)
                mm_f16(psA, psB, mt, 0, start=True, stop=False)
                for kt2 in range(KT8):
                    mm_f8(psA, psB, mt, kt2)
                for kt in range(1, KT16):
                    mm_f16(psA, psB, mt, kt, start=False, stop=(kt == KT16 - 1))
                evict(psA, mt, 0, N0_W)
                evict(psB, mt, N0_W, N1_W)

            # ---- phase 2 tail: last mt fully group-serial; the N0 chunk's
            # eviction overlaps the N1 chunk's matmuls, and the final N1
            # eviction is sliced so only ~1us trails the last matmul.
            mt = MT - 1
            psA, psB = ps_pair(mt)
            nc.tensor.matmul(
                psA[:, :N0_W],
                lhsT=x16_sb[:, 0, mt * P : (mt + 1) * P],
                rhs=w16_sb[:, 0, 0:N0_W],
                start=True,
                stop=False,
            )
            for kt2 in range(KT8):
                lhsT = x8_sb[:, kt2, :, mt * P : (mt + 1) * P]
                for n0, cw in F8_CHUNKS_A:
                    nc.tensor.matmul(
                        psA[:, n0 : n0 + cw],
                        lhsT=lhsT,
                        rhs=w8_sb[:, kt2, :, n0 : n0 + cw],
                        start=False,
                        stop=False,
                        perf_mode=DR,
                    )
            for kt in range(1, KT16):
                nc.tensor.matmul(
                    psA[:, :N0_W],
                    lhsT=x16_sb[:, kt, mt * P : (mt + 1) * P],
                    rhs=w16_sb[:, kt, 0:N0_W],
                    start=False,
                    stop=(kt == KT16 - 1),
                )
            evict(psA, mt, 0, N0_W, slices=2)
            nc.tensor.matmul(
                psB[:, :N1_W],
                lhsT=x16_sb[:, 0, mt * P : (mt + 1) * P],
                rhs=w16_sb[:, 0, N0_W:N],
                start=True,
                stop=False,
            )
            for kt2 in range(KT8):
                lhsT = x8_sb[:, kt2, :, mt * P : (mt + 1) * P]
                for n0, cw in F8_CHUNKS_B:
                    nc.tensor.matmul(
                        psB[:, n0 - N0_W : n0 - N0_W + cw],
                        lhsT=lhsT,
                        rhs=w8_sb[:, kt2, :, n0 : n0 + cw],
                        start=False,
                        stop=False,
                        perf_mode=DR,
                    )
            # bias for the final group rides a 1-partition ones-row matmul so
            # the eviction can skip the vector add and DMA straight from
            # PSUM -- only the direct store trails the last matmul.
            nc.tensor.matmul(
                psB[:, :N1_W],
                lhsT=ones_t[0:1, 0:P],
                rhs=bias16_t[0:1, N0_W:N],
                start=False,
                stop=False,
            )
            for kt in range(1, KT16):
                nc.tensor.matmul(
                    psB[:, :N1_W],
                    lhsT=x16_sb[:, kt, mt * P : (mt + 1) * P],
                    rhs=w16_sb[:, kt, N0_W:N],
                    start=False,
                    stop=(kt == KT16 - 1),
                )
            # Parallel eviction: scalar (activation Copy) and vector each move
            # half the PSUM to SBUF simultaneously, then store on separate
            # DGE queues.
            half = N1_W // 2
            otB = opool.tile([P, N0_W], f32, tag="ot", name="ot_final")
            nc.scalar.activation(
                otB[:, 0:half], psB[:, 0:half], mybir.ActivationFunctionType.Copy
            )
            nc.sync.dma_start(out_r[mt, :, N0_W : N0_W + half], otB[:, 0:half])
            nc.vector.tensor_copy(otB[:, half:N1_W], psB[:, half:N1_W])
            nc.scalar.dma_start(out_r[mt, :, N0_W + half : N], otB[:, half:N1_W])

    nc.compile()
    return nc


def _get_nc():
    if "nc" not in _NC_CACHE:
        _NC_CACHE["nc"] = _build_nc()
    return _NC_CACHE["nc"]


def _run(in_maps, trace=False, **kwargs):
    from concourse.bass_utils import run_bass_kernel_spmd

    nc = _get_nc()
    return run_bass_kernel_spmd(
        nc, in_maps, core_ids=list(range(N_CORES)), trace=trace, **kwargs
    )


def _interleave_f8(aT):
    """[K1, cols] K-major fp8 -> [KT8, P, 2*cols] DoubleRow layout:
    out[kt2][p][i*cols + c] = aT[kt2*256 + i*128 + p, c]."""
    cols = aT.shape[1]
    return np.ascontiguousarray(
        aT.reshape(KT8, 2, P, cols).transpose(0, 2, 1, 3).reshape(KT8, P, 2 * cols)
    )


def _make_in_maps(x, W, b):
    import ml_dtypes

    e4m3 = ml_dtypes.float8_e4m3
    x = np.asarray(x, dtype=np.float32)
    W = np.asarray(W, dtype=np.float32)
    b = np.asarray(b, dtype=np.float32)

    w8 = (W[:, :K1].T * np.float32(SW8)).astype(e4m3)  # [K1, N]
    w16 = (W[:, K1:].T * np.float32(SCALE)).astype(np.float16)  # [K2, N]
    w8T = _interleave_f8(w8)
    bias = np.ascontiguousarray(
        np.broadcast_to((b * np.float32(SCALE))[None, :], (P, N))
    )
    bias16 = (b * np.float32(SCALE)).astype(np.float16).reshape(1, N)
    x8_full = (x[:, :K1].T * np.float32(SX)).astype(e4m3)  # [K1, B]
    x16_full = np.ascontiguousarray(x[:, K1:].T).astype(np.float16)  # [K2, B]
    return [
        {
            "x8T": _interleave_f8(x8_full[:, c * M : (c + 1) * M]),
            "x16T": np.ascontiguousarray(x16_full[:, c * M : (c + 1) * M]),
            "w8T": w8T,
            "w16T": w16,
            "bias": bias,
            "bias16": bias16,
        }
        for c in range(N_CORES)
    ]


def kernel(x, W, b):
    res = _run(_make_in_maps(x, W, b))
    return np.concatenate(
        [r["out"].astype(np.float32) * INV_SCALE for r in res.results], axis=0
    )


# revision 20
# speedup vs baseline: 1.1563x; 1.0071x over previous
"""Trainium2 Bass kernel for nn_HRNetW30classifier: logits = x @ W.T + b.

Shapes (full): x (8192, 2048) f32, W (1000, 2048) f32, b (1000,) f32
Output: (8192, 1000) f32.

Sharding: data-parallel over batch across 8 NeuronCores. Each core computes a
(1024, 2048) @ (2048, 1000) GEMM with W/b replicated.

Numerics: hybrid fp8/fp16. Features [0:K1=512] run as e4m3 DoubleRow matmuls
(2 k-rows per partition, 2x PE throughput); features [K1:2048] run in fp16.
Host pre-scales so one PSUM accumulation carries both parts at a common 256x
scale (x8 = e4m3(4x), w8 = e4m3(64W), w16 = fp16(256W) -- all pow-2 exact),
and the final /256 is an exact host-side multiply after the gather. Measured
rel err vs the fp32 reference: 1.54e-2 (fp16-only: 2.4e-4).

Schedule (from NTFF trace analysis):
- Heavy warmups: full 128-partition x 256-col dummy matmuls from t~6us open
  the HAM clock window ~5.8us later, so real matmuls run at the full
  2.4GHz (0.411 ns/col) instead of the ~2GHz a cold PE sustains.
- Inputs ride BOTH hardware DGE queues (Sync: W, Activation: x) in
  need-order: fp8 k-slices first, then per-kt fp16 phase-1 halves, then the
  batched phase-2 half.
- Phase 1 (m-tiles 0-3) runs k-outer paced by the stream; phase 2 runs
  group-serial per m-tile so evictions stagger; the last m-tile runs its two
  N-chunks serially with sliced evictions alternating the two DGE queues so
  only ~1us trails the final matmul.
"""

import numpy as np

P = 128
N_CORES = 8
B_FULL = 8192
M = B_FULL // N_CORES  # 1024 batch rows per core
N = 1000  # classes
K = 2048  # features
K1 = 512  # features computed in fp8 DoubleRow
K2 = K - K1  # 1536 features computed in fp16
KT8 = K1 // (2 * P)  # 2 double-k-tiles
KT16 = K2 // P  # 12 fp16 k-tiles
MT = M // P  # 8 m-tiles
MH = MT // 2  # 4 m-tiles per phase
MH_COLS = MH * P  # 512 x columns needed for phase 1
N0_W = 512  # first n-chunk (one PSUM bank of fp32)
N1_W = N - N0_W  # 488
SX = 4.0  # host scale on x before e4m3 quantization
SW8 = 64.0  # host scale on W[:, :K1] before e4m3 quantization
SCALE = SX * SW8  # PSUM carries SCALE * logits
INV_SCALE = np.float32(1.0 / SCALE)  # exact pow-2 host dequant

WARM_N = 15  # heavy warmup matmuls: bridge PE activity from preamble end
# (~7.2us, ~214ns each) until the first k-slice lands (~10us). The HAM
# full-clock window opens after ~4us of CONTINUOUS heavy activity; any PE
# idle gap before real matmuls restarts that ramp (costs ~2.5us), so err on
# the long side -- extra warmups only cost ~0.2us each.

_NC_CACHE = {}


def _build_nc():
    """Build + compile the per-core Bass program (SPMD: same NEFF on 8 cores)."""
    from contextlib import ExitStack

    import concourse.tile as tile
    from concourse import bacc, mybir
    from concourse._compat import get_trn_type

    f32 = mybir.dt.float32
    f16 = mybir.dt.float16
    f8 = mybir.dt.float8e4
    DR = mybir.MatmulPerfMode.DoubleRow

    nc = bacc.Bacc(get_trn_type() or "TRN2", target_bir_lowering=False, debug=False)

    # fp8 operands, host-interleaved for DoubleRow: [kt2][p][i*cols + c] holds
    # feature k = kt2*256 + i*128 + p.
    x8T = nc.dram_tensor("x8T", [KT8, P, 2 * M], f8, kind="ExternalInput")
    w8T = nc.dram_tensor("w8T", [KT8, P, 2 * N], f8, kind="ExternalInput")
    # fp16 operands for features K1..K, K-major.
    x16T = nc.dram_tensor("x16T", [K2, M], f16, kind="ExternalInput")
    w16T = nc.dram_tensor("w16T", [K2, N], f16, kind="ExternalInput")
    bias = nc.dram_tensor("bias", [P, N], f32, kind="ExternalInput")  # SCALE*b
    out = nc.dram_tensor("out", [M, N], f32, kind="ExternalOutput")  # SCALE*logits

    x16_r = x16T.ap().rearrange("(kt p) m -> kt p m", p=P)  # [KT16, 128, M]
    x16_p = x16T.ap().rearrange("(kt p) m -> p kt m", p=P)  # [128, KT16, M]
    w16_r = w16T.ap().rearrange("(kt p) n -> kt p n", p=P)  # [KT16, 128, N]
    out_r = out.ap().rearrange("(mt p) n -> mt p n", p=P)  # [MT, 128, N]

    with tile.TileContext(nc) as tc:
        with ExitStack() as ctx:
            xpool = ctx.enter_context(tc.tile_pool(name="xpool", bufs=1))
            wpool = ctx.enter_context(tc.tile_pool(name="wpool", bufs=1))
            bpool = ctx.enter_context(tc.tile_pool(name="bpool", bufs=1))
            opool = ctx.enter_context(tc.tile_pool(name="opool", bufs=8))
            pspool = ctx.enter_context(tc.tile_pool(name="ps", bufs=8, space="PSUM"))

            x8_sb = xpool.tile([P, KT8, 2, M], f8, tag="x8")
            x16_sb = xpool.tile([P, KT16, M], f16, tag="x16")
            w8_sb = wpool.tile([P, KT8, 2, N], f8, tag="w8")
            w16_sb = wpool.tile([P, KT16, N], f16, tag="w16")
            warm = bpool.tile([P, 384], f16, tag="warm")
            bias_t = bpool.tile([P, N], f32, tag="bias")

            # Input DMA streams on BOTH hardware DGE queues, in need-order.
            # Sync queue: W (fp8 slices then fp16 k-slices, 2000B lines).
            # Activation queue: x (fp8 slices, fp16 phase-1 halves, bias,
            # then the whole fp16 phase-2 half as one batched 3D DMA).
            # fp16 kt=0 leads (it opens every PSUM bank with start=True),
            # then the fp8 slices, then the remaining fp16 k-slices.
            nc.scalar.dma_start(x16_sb[:, 0, 0:P], x16_r[0][:, 0:P])
            nc.sync.dma_start(w16_sb[:, 0, 0:N0_W], w16_r[0][:, 0:N0_W])
            nc.scalar.dma_start(x16_sb[:, 0, P:MH_COLS], x16_r[0][:, P:MH_COLS])
            nc.sync.dma_start(w16_sb[:, 0, N0_W:N], w16_r[0][:, N0_W:N])
            for kt2 in range(KT8):
                nc.scalar.dma_start(x8_sb[:, kt2, :, :], x8T.ap()[kt2])
                nc.sync.dma_start(w8_sb[:, kt2, :, :], w8T.ap()[kt2])
            for kt in range(1, KT16):
                nc.sync.dma_start(w16_sb[:, kt, :], w16_r[kt])
                nc.scalar.dma_start(x16_sb[:, kt, 0:MH_COLS], x16_r[kt][:, 0:MH_COLS])
                if kt == 3:
                    # bias rides early-mid stream: needed by the first
                    # evictions (~35us).
                    nc.scalar.dma_start(bias_t[:], bias.ap())
            # fp16 phase-2 x halves: one 1.5MB batched DMA.
            nc.scalar.dma_start(x16_sb[:, :, MH_COLS:M], x16_p[:, :, MH_COLS:M])

            # Heavy warmups: full 128-partition stationary tile and 256-col
            # moving tile, so the HAM activity monitor sees real PE load and
            # opens the full-clock window before the first data-dependent
            # matmul. Results go to a scratch PSUM bank that is never read.
            nc.gpsimd.memset(warm[:], 0.7071)
            ps_w = pspool.tile([P, N0_W], f32, tag="ps", name="ps_warm")
            for _ in range(WARM_N):
                nc.tensor.matmul(
                    ps_w[:, :256],
                    lhsT=warm[:, 0:P],
                    rhs=warm[:, P:384],
                    start=True,
                    stop=True,
                )

            # fp8 n-chunks: DoubleRow moving free dim is 2*cw <= 512.
            F8_CHUNKS_A = [(0, 256), (256, 256)]  # psA covers n 0:512
            F8_CHUNKS_B = [(512, 256), (768, 232)]  # psB covers n 512:1000

            # HW start=True zeroes PSUM at bank granularity (wider than the
            # instruction's write region), so each bank must be opened by
            # exactly one full-width matmul: the fp16 kt=0 chunk. All fp8
            # DoubleRow chunks accumulate with start=False.
            def mm_f8(psA, psB, mt, kt2):
                lhsT = x8_sb[:, kt2, :, mt * P : (mt + 1) * P]
                for n0, cw in F8_CHUNKS_A:
                    nc.tensor.matmul(
                        psA[:, n0 : n0 + cw],
                        lhsT=lhsT,
                        rhs=w8_sb[:, kt2, :, n0 : n0 + cw],
                        start=False,
                        stop=False,
                        perf_mode=DR,
                    )
                for n0, cw in F8_CHUNKS_B:
                    nc.tensor.matmul(
                        psB[:, n0 - N0_W : n0 - N0_W + cw],
                        lhsT=lhsT,
                        rhs=w8_sb[:, kt2, :, n0 : n0 + cw],
                        start=False,
                        stop=False,
                        perf_mode=DR,
                    )

            def mm_f16(psA, psB, mt, kt, start, stop):
                lhsT = x16_sb[:, kt, mt * P : (mt + 1) * P]
                nc.tensor.matmul(
                    psA[:, :N0_W],
                    lhsT=lhsT,
                    rhs=w16_sb[:, kt, 0:N0_W],
                    start=start,
                    stop=stop,
                )
                nc.tensor.matmul(
                    psB[:, :N1_W],
                    lhsT=lhsT,
                    rhs=w16_sb[:, kt, N0_W:N],
                    start=start,
                    stop=stop,
                )

            _evict_i = [0]

            def evict(ps_t, mt, n0, nw, slices=1):
                """Bias-add + store, optionally sliced for tail pipelining.
                Adds run on vector (gpsimd cannot read PSUM); DMAs alternate
                the two DGE queues so the tail drains in parallel."""
                ot = opool.tile([P, N0_W], f32, tag="ot", name=f"ot_{n0}_{mt}")
                step = -(-nw // slices)
                for s0 in range(0, nw, step):
                    sw = min(step, nw - s0)
                    i = _evict_i[0]
                    _evict_i[0] += 1
                    dma_eng = nc.sync if i % 2 == 0 else nc.scalar
                    nc.vector.tensor_add(
                        ot[:, s0 : s0 + sw],
                        ps_t[:, s0 : s0 + sw],
                        bias_t[:, n0 + s0 : n0 + s0 + sw],
                    )
                    dma_eng.dma_start(
                        out_r[mt, :, n0 + s0 : n0 + s0 + sw], ot[:, s0 : s0 + sw]
                    )

            def ps_pair(mt):
                a = pspool.tile([P, N0_W], f32, tag="ps", name=f"psA_{mt}")
                b = pspool.tile([P, N0_W], f32, tag="ps", name=f"psB_{mt}")
                return a, b

            # ---- phase 1: mt 0..3, k-outer, paced by the DMA stream ----
            ps1 = [ps_pair(mt) for mt in range(MH)]
            for mt in range(MH):
                mm_f16(*ps1[mt], mt, 0, start=True, stop=False)
            for kt2 in range(KT8):
                for mt in range(MH):
                    mm_f8(*ps1[mt], mt, kt2)
            for kt in range(1, KT16):
                for mt in range(MH):
                    mm_f16(*ps1[mt], mt, kt, start=False, stop=(kt == KT16 - 1))
            for mt in range(MH):
                evict(ps1[mt][0], mt, 0, N0_W)
                evict(ps1[mt][1], mt, N0_W, N1_W)

            # ---- phase 2: mt 4..6 group-serial so evictions stagger ----
            for mt in range(MH, MT - 1):
                psA, psB = ps_pair(mt)
                mm_f16(psA, psB, mt, 0, start=True, stop=False)
                for kt2 in range(KT8):
                    mm_f8(psA, psB, mt, kt2)
                for kt in range(1, KT16):
                    mm_f16(psA, psB, mt, kt, start=False, stop=(kt == KT16 - 1))
                evict(psA, mt, 0, N0_W)
                evict(psB, mt, N0_W, N1_W)

            # ---- phase 2 tail: last mt in THREE serial groups (512/360/128
            # cols). Each group's eviction overlaps the next group's matmuls;
            # only the tiny 128-col eviction trails the final matmul, so the
            # tail is add + one small DMA (~1.7us) instead of a 488-col
            # eviction (~2.7us).
            mt = MT - 1

            def tail_group(ps_t, n0, nw, f8chunks, slices):
                nc.tensor.matmul(
                    ps_t[:, :nw],
                    lhsT=x16_sb[:, 0, mt * P : (mt + 1) * P],
                    rhs=w16_sb[:, 0, n0 : n0 + nw],
                    start=True,
                    stop=False,
                )
                for kt2 in range(KT8):
                    lhsT = x8_sb[:, kt2, :, mt * P : (mt + 1) * P]
                    for c0, cw in f8chunks:
                        nc.tensor.matmul(
                            ps_t[:, c0 - n0 : c0 - n0 + cw],
                            lhsT=lhsT,
                            rhs=w8_sb[:, kt2, :, c0 : c0 + cw],
                            start=False,
                            stop=False,
                            perf_mode=DR,
                        )
                for kt in range(1, KT16):
                    nc.tensor.matmul(
                        ps_t[:, :nw],
                        lhsT=x16_sb[:, kt, mt * P : (mt + 1) * P],
                        rhs=w16_sb[:, kt, n0 : n0 + nw],
                        start=False,
                        stop=(kt == KT16 - 1),
                    )
                evict(ps_t, mt, n0, nw, slices=slices)

            psA, psB = ps_pair(mt)
            psC = pspool.tile([P, N0_W], f32, tag="ps", name="psC_7")
            tail_group(psA, 0, 512, [(0, 256), (256, 256)], slices=2)
            tail_group(psB, 512, 360, [(512, 256), (768, 104)], slices=2)
            tail_group(psC, 872, 128, [(872, 128)], slices=1)

    nc.compile()
    return nc


def _get_nc():
    if "nc" not in _NC_CACHE:
        _NC_CACHE["nc"] = _build_nc()
    return _NC_CACHE["nc"]


def _run(in_maps, trace=False, **kwargs):
    from concourse.bass_utils import run_bass_kernel_spmd

    nc = _get_nc()
    return run_bass_kernel_spmd(
        nc, in_maps, core_ids=list(range(N_CORES)), trace=trace, **kwargs
    )


def _interleave_f8(aT):
    """[K1, cols] K-major fp8 -> [KT8, P, 2*cols] DoubleRow layout:
    out[kt2][p][i*cols + c] = aT[kt2*256 + i*128 + p, c]."""
    cols = aT.shape[1]
    return np.ascontiguousarray(
        aT.reshape(KT8, 2, P, cols).transpose(0, 2, 1, 3).reshape(KT8, P, 2 * cols)
    )


def _make_in_maps(x, W, b):
    import ml_dtypes

    e4m3 = ml_dtypes.float8_e4m3
    x = np.asarray(x, dtype=np.float32)
    W = np.asarray(W, dtype=np.float32)
    b = np.asarray(b, dtype=np.float32)

    w8 = (W[:, :K1].T * np.float32(SW8)).astype(e4m3)  # [K1, N]
    w16 = (W[:, K1:].T * np.float32(SCALE)).astype(np.float16)  # [K2, N]
    w8T = _interleave_f8(w8)
    bias = np.ascontiguousarray(
        np.broadcast_to((b * np.float32(SCALE))[None, :], (P, N))
    )
    x8_full = (x[:, :K1].T * np.float32(SX)).astype(e4m3)  # [K1, B]
    x16_full = np.ascontiguousarray(x[:, K1:].T).astype(np.float16)  # [K2, B]
    return [
        {
            "x8T": _interleave_f8(x8_full[:, c * M : (c + 1) * M]),
            "x16T": np.ascontiguousarray(x16_full[:, c * M : (c + 1) * M]),
            "w8T": w8T,
            "w16T": w16,
            "bias": bias,
        }
        for c in range(N_CORES)
    ]


def kernel(x, W, b):
    res = _run(_make_in_maps(x, W, b))
    return np.concatenate(
        [r["out"].astype(np.float32) * INV_SCALE for r in res.results], axis=0
    )
